# revision 30
# baseline (speedup 1.0000x reference)
"""Causal attention product kernel for Trainium2, SPMD over 8 NeuronCores.

Math (faithful to the nn.Module reference):
    scores = (Q @ K^T) / 8 + mask          [B,H,S,S], mask is [B,1,1,S]
    scores[..., -128:, -128:] = tril(ones,-1).T * finfo.min   (overwrite!)
    out = softmax(scores, -1) @ V

Sharding: B*H = 24 heads split 3-per-core across 8 cores; no cross-core
communication.

Per-core algorithm (per head), flash-attention style -- the [S,S] score
matrix never hits DRAM:
  - Host pre-transposes Q,K to [128, S] bf16 (zero-padded from d=64 so the
    S^T weights span 128 partitions and fast-weight-load applies) and
    pre-scales V rows by exp(mask_k), appending an exp(mask_k) column that
    accumulates the softmax denominator and a zero pad column.
    exp(s + m) = exp(s)*exp(m) makes the additive mask exact while keeping
    the matmul contraction free of a bias row.
  - For each 512-query block: S^T tiles [128k, 512q] on PE (bf16 in, fp32
    psum), exp on ACT (psum -> bf16 sbuf, the 1/8 scale fused into the
    activation), PV matmul with V-stationary [128, 66] bf16 weights
    accumulating OUT^T [66, 512] in PSUM.  PE-transpose back to [q, d],
    divide by the denominator on DVE, DMA out.
  - The (head, q-block) units are software-pipelined: unit i's S^T/exp
    chunks interleave with unit i-1's ACT-paced PV matmuls so the PE
    instruction stream stays dense and the HAM clock gate keeps the array
    at 2.4 GHz (a bursty stream re-throttles it to 1.2 GHz and the kernel
    becomes cold-PE-bound).
  - The overwritten bottom-right 128x128 block of probs is exactly
    tril(ones) * exp(-mask_k) (so the V pre-scale cancels): DMA'd from the
    host straight over P^T before the PV matmul.

Measured on trn2 (8 cores): ~403 us HW exec, rel-L2 error ~3.5e-3 vs the
fp32 reference (dominated by bf16 rounding of Q,K,V and P).
"""

import os
import sys

for _p in ("/opt/trn_rl_repo", "/root/.axon_site/_ro/trn_rl_repo"):
    if os.path.isdir(_p) and _p not in sys.path:
        sys.path.insert(0, _p)

import ml_dtypes
import numpy as np

import concourse.bass as bass
import concourse.mybir as mybir
import concourse.tile as tile
from concourse import bacc
from concourse import bass_utils

B, H, S, D = 2, 12, 4096, 64
N_CORES = 8
HPC = (B * H) // N_CORES  # heads per core = 3

KTILES = S // 128  # 32 k-tiles of 128
QBS = 512          # queries per block
QB = S // QBS      # 8 query blocks
CH = 3             # k-tiles per ACT chunk (3 psum banks per S^T tile)

F32R = mybir.dt.float32r
F32 = mybir.dt.float32
BF16 = mybir.dt.bfloat16


def _chunks():
    out = []
    kt = 0
    while kt < KTILES:
        n = min(CH, KTILES - kt)
        out.append((kt, n))
        kt += n
    return out


def _kernel_body(tc, q_d, k_d, v_d, ut_d, ident_d, o_d):
    nc = tc.nc

    singles = tc.alloc_tile_pool(name="singles", bufs=1)
    qkpool = tc.alloc_tile_pool(name="qk", bufs=2)
    vpool = tc.alloc_tile_pool(name="v", bufs=2)
    ptpool = tc.alloc_tile_pool(name="pt", bufs=2)
    otpool = tc.alloc_tile_pool(name="ot", bufs=2)
    outpool = tc.alloc_tile_pool(name="outsb", bufs=3)
    rpool = tc.alloc_tile_pool(name="r", bufs=4)
    spsum = tc.alloc_tile_pool(name="spsum", bufs=2, space="PSUM")
    opsum = tc.alloc_tile_pool(name="opsum", bufs=2, space="PSUM")

    # Identity for the output PE transposes (fp32r path).
    ident = singles.tile([128, 128], F32R, name="ident")
    nc.sync.dma_start(out=ident, in_=ident_d)

    # Prime slow one-time state while the first head's DMAs stream:
    #  - a throwaway exp pulls the ~2.7us ACT table load off the critical
    #    path;
    #  - ~3.4us of throwaway matmuls keep the PE busy through one HAM
    #    activity window so the real S^T chunks start at 2.4 GHz instead
    #    of paying the cold-clock ramp.
    warm_sb = singles.tile([128, 128], BF16, name="warm_sb")
    nc.vector.memset(warm_sb[:, 0:2], 0.0)
    nc.scalar.activation(
        out=warm_sb[:, 0:2],
        in_=warm_sb[:, 0:2],
        func=mybir.ActivationFunctionType.Exp,
        scale=0.125,
    )
    sp_warm = spsum.tile([128, CH, QBS], F32, name="sp")
    for _ in range(16):
        nc.tensor.matmul(
            sp_warm[:, 0, 0:128], lhsT=ident, rhs=ident, start=True, stop=True
        )

    # Software pipeline over (head, q-block) units: while unit i's S^T
    # chunks stream through PE->ACT, unit i-1's ACT-paced PV matmuls fill
    # the PE gaps.  A bursty PE stream lets the HAM clock gate re-throttle
    # the array to 1.2 GHz; this interleave keeps PE density high and the
    # array at 2.4 GHz.
    units = [(h, qb) for h in range(HPC) for qb in range(QB)]
    heads = {}
    prev = None  # (h, qb, pt, vt)

    def emit_pv_chunk(pv, kt0, nch):
        h_p, qb_p, pt_p, vt_p, op_p = pv
        for kt in range(kt0, kt0 + nch):
            nc.tensor.matmul(
                op_p[0 : D + 2, :],
                lhsT=vt_p[:, kt, :],
                rhs=pt_p[:, kt, :],
                start=(kt == 0),
                stop=(kt == KTILES - 1),
            )

    def emit_out(pv):
        h_p, qb_p, pt_p, vt_p, op_p = pv
        # 66 rows (row 65 is V's zero pad column) so the 66-wide transpose
        # identity is a true permutation; fp32r matmuls need an even
        # innermost count.
        ot = otpool.tile([66, QBS], F32R, name="ot")
        nc.vector.tensor_copy(out=ot, in_=op_p[0:66, :])
        osb = outpool.tile([128, 4, D], F32, name="osb")
        tp = opsum.tile([128, QBS], F32R, name="tp", tag="o")
        for sub in range(4):
            nc.tensor.transpose(
                tp[:, sub * 128 : sub * 128 + 66],
                ot[:, sub * 128 : (sub + 1) * 128],
                ident[0:66, 0:66],
            )
            r = rpool.tile([128, 1], F32, name="r")
            nc.vector.reciprocal(r, tp[:, sub * 128 + 64 : sub * 128 + 65])
            nc.vector.tensor_scalar_mul(
                osb[:, sub, :], tp[:, sub * 128 : sub * 128 + 64], r[:, 0:1]
            )
        nc.sync.dma_start(
            out=o_d[h_p, qb_p * QBS : (qb_p + 1) * QBS, :].rearrange(
                "(s p) d -> p s d", p=128
            ),
            in_=osb,
        )

    def load_head(h):
        # ---- load pre-transposed Q^T, K^T and pre-scaled V' ----
        # Queue order matters for head 0 (the pipeline ramp is DMA-paced):
        # unit (0,0) reads ALL of ktt but only qt's first 512 columns, so
        # push ktt pieces and qt[0] first, then qt[1] (unit (0,1)), then V
        # (first PV), then the rest of qt.
        qt = qkpool.tile([128, S], BF16, name="qt")
        ktt = qkpool.tile([128, S], BF16, name="ktt")
        vt = vpool.tile([128, KTILES, D + 2], BF16, name="vt")

        def load_q(g):
            cols = slice(g * 512, (g + 1) * 512)
            nc.sync.dma_start(out=qt[:, cols], in_=q_d[h, :, cols])

        def load_k(g):
            cols = slice(g * 512, (g + 1) * 512)
            nc.sync.dma_start(out=ktt[:, cols], in_=k_d[h, :, cols])

        def load_v(g):
            nc.sync.dma_start(
                out=vt[:, g * 4 : (g + 1) * 4, :],
                in_=v_d[h, g * 512 : (g + 1) * 512, :].rearrange(
                    "(c p) f -> p c f", p=128
                ),
            )

        load_k(0)
        load_q(0)
        for g in range(1, 8):
            load_k(g)
        load_q(1)
        for g in range(8):
            load_v(g)
        for g in range(2, 8):
            load_q(g)
        heads[h] = (qt, ktt, vt)

    for ui, (h, qb) in enumerate(units):
        is_last = ui == len(units) - 1
        if qb == 0 and h == 0:
            load_head(0)
        if qb == QB - 2 and h + 1 < HPC:
            # prefetch the next head's operands so its first S^T chunk
            # doesn't stall on DMA at the head boundary
            load_head(h + 1)
        qt, ktt, vt = heads[h]

        qs = slice(qb * QBS, (qb + 1) * QBS)
        pt = ptpool.tile([128, KTILES, QBS], BF16, name="pt")
        if prev is not None:
            op = opsum.tile([128, QBS], F32, name="op", tag="o")
            prev = prev + (op,)
        cur = None
        if is_last:
            # the final unit has no successor to hide its PV matmuls under;
            # start consuming its own P^T inside the chunk loop (two chunks
            # behind, so the exp results are always ready) to shorten the
            # serial epilogue
            op_last = opsum.tile([128, QBS], F32, name="op", tag="o")
            cur = (h, qb, pt, vt, op_last)
        absorbed = 0
        for ci, (kt0, nch) in enumerate(_chunks()):
            sp = spsum.tile([128, CH, QBS], F32, name="sp")
            for i in range(nch):
                kt = kt0 + i
                nc.tensor.matmul(
                    sp[:, i, :],
                    lhsT=ktt[:, kt * 128 : (kt + 1) * 128],
                    rhs=qt[:, qs],
                    start=True,
                    stop=True,
                )
            nc.scalar.activation(
                out=pt[:, kt0 : kt0 + nch, :],
                in_=sp[:, 0:nch, :],
                func=mybir.ActivationFunctionType.Exp,
                scale=0.125,
            )
            if prev is not None:
                emit_pv_chunk(prev, kt0, nch)
            if cur is not None and ci >= 2:
                emit_pv_chunk(cur, absorbed, 1)
                absorbed += 1
        if qb == QB - 1:
            # overwrite probs of the bottom-right 128x128 block with the
            # host-computed tril(ones)*exp(-mask) tile
            nc.sync.dma_start(out=pt[:, KTILES - 1, QBS - 128 : QBS], in_=ut_d[h])
        if prev is not None:
            emit_out(prev)
        prev = (h, qb, pt, vt) if cur is None else cur

    # epilogue: remaining PV + output for the final unit
    for kt in range(absorbed, KTILES):
        emit_pv_chunk(prev, kt, 1)
    emit_out(prev)

    for pool in (opsum, spsum, rpool, outpool, otpool, ptpool, vpool, qkpool, singles):
        pool.release()


_CACHED = None


def _build():
    global _CACHED
    if _CACHED is not None:
        return _CACHED
    nc = bacc.Bacc(trn_type="TRN2", target_bir_lowering=False, debug=False)
    q_d = nc.dram_tensor("q", [HPC, 128, S], BF16, kind="ExternalInput").ap()
    k_d = nc.dram_tensor("k", [HPC, 128, S], BF16, kind="ExternalInput").ap()
    v_d = nc.dram_tensor("v", [HPC, S, D + 2], BF16, kind="ExternalInput").ap()
    ut_d = nc.dram_tensor("ut", [HPC, 128, 128], BF16, kind="ExternalInput").ap()
    ident_d = nc.dram_tensor("ident", [128, 128], F32R, kind="ExternalInput").ap()
    o_d = nc.dram_tensor("o", [HPC, S, D], F32, kind="ExternalOutput").ap()
    with tile.TileContext(nc) as tc:
        _kernel_body(tc, q_d, k_d, v_d, ut_d, ident_d, o_d)
    nc.compile()
    _CACHED = nc
    return nc


def _shard_inputs(query_layer, key_layer, value_layer, attention_mask):
    q = np.asarray(query_layer, dtype=np.float32).reshape(B * H, S, D)
    k = np.asarray(key_layer, dtype=np.float32).reshape(B * H, S, D)
    v = np.asarray(value_layer, dtype=np.float32).reshape(B * H, S, D)
    m = np.asarray(attention_mask, dtype=np.float32).reshape(B, S)
    m_heads = np.repeat(m, H, axis=0)  # [B*H, S]

    # zero-pad the contraction dim to 128 rows: identical math, but gives
    # the S^T weights 128 partitions so fast-weight-load kicks in on PE
    qt = np.zeros((B * H, 128, S), dtype=ml_dtypes.bfloat16)
    qt[:, :D, :] = q.transpose(0, 2, 1).astype(ml_dtypes.bfloat16)
    kt = np.zeros((B * H, 128, S), dtype=ml_dtypes.bfloat16)
    kt[:, :D, :] = k.transpose(0, 2, 1).astype(ml_dtypes.bfloat16)

    # V' = [V * exp(m_k) | exp(m_k)]; the mask rides along multiplicatively
    # and the appended column accumulates the softmax denominator.
    em = np.exp(np.clip(m_heads, -6e4, 60.0))[:, :, None]  # [B*H, S, 1]
    zc = np.zeros_like(em)
    vs = np.concatenate([v * em, em, zc], axis=2).astype(ml_dtypes.bfloat16)  # [B*H,S,66]

    # P^T overwrite tile for the bottom-right block: tril(ones).T in P^T
    # layout times exp(-m) so the V' pre-scale cancels exactly.
    tri = (np.arange(128)[:, None] <= np.arange(128)[None, :]).astype(np.float32)
    inv_em = np.where(em[:, -128:, 0] > 0.0, 1.0 / np.maximum(em[:, -128:, 0], 1e-37), 0.0)
    ut = (tri[None, :, :] * inv_em[:, :, None]).astype(ml_dtypes.bfloat16)  # [B*H,128,128]

    ident = np.eye(128, dtype=np.float32)

    in_maps = []
    for c in range(N_CORES):
        hs = slice(c * HPC, (c + 1) * HPC)
        in_maps.append(
            {
                "q": np.ascontiguousarray(qt[hs]),
                "k": np.ascontiguousarray(kt[hs]),
                "v": np.ascontiguousarray(vs[hs]),
                "ut": np.ascontiguousarray(ut[hs]),
                "ident": ident,
            }
        )
    return in_maps


def run(query_layer, key_layer, value_layer, attention_mask, trace=False):
    """Build + run on 8 cores; returns (full_output, BassKernelResults)."""
    nc = _build()
    in_maps = _shard_inputs(query_layer, key_layer, value_layer, attention_mask)
    res = bass_utils.run_bass_kernel_spmd(
        nc, in_maps, core_ids=list(range(N_CORES)), trace=trace
    )
    out = np.concatenate(
        [res.results[c]["o"].reshape(HPC, S, D) for c in range(N_CORES)], axis=0
    )
    return out.reshape(B, H, S, D).astype(np.float32), res


def kernel(query_layer, key_layer, value_layer, attention_mask):
    out, _ = run(query_layer, key_layer, value_layer, attention_mask)
    return out
